# revision 19
# baseline (speedup 1.0000x reference)
"""Equidistant DISCO conv2d (effectively a 5x5 conv, 32->32 ch, 4x1024x1024)
on 8 trn2 NeuronCores.

Sharding: core c handles (batch b = c//2, image half hh = c%2) -> 512 rows.
Inside a core the 512 output rows are split into 4 slabs of 128 rows packed
into SBUF partition groups (partition p = 32*slab + in_ch), so each tap of
the conv runs as 16 concurrent 32x32 matmuls via PE tile_position packing
(4 row-groups = slabs x 4 col-groups = output rows), accumulating 25 taps
into PSUM. bf16 operands, fp32 PSUM accumulation.

The 7x7 DISCO kernel's outer ring is exactly zero (radial hat function hits
zero at r == cutoff), so only the inner 5x5 = 25 taps are computed.
"""

import sys

sys.path.insert(0, "/opt/trn_rl_repo")

import ml_dtypes
import numpy as np

import concourse.bacc as bacc
import concourse.tile as tile
from concourse import mybir
from concourse.bass_utils import run_bass_kernel_spmd

# ---- static geometry ----
B, CH, H, W = 4, 32, 1024, 1024
N_CORES = 8
HALF = H // 2          # rows per core
SLABS = 4
SLAB_ROWS = HALF // SLABS   # 128
PAD = 2                # effective 5x5 kernel
KS = 5
TAPS = [(dy, dx) for dy in range(KS) for dx in range(KS)]
NT = len(TAPS)         # 25
WP = W + 2 * PAD       # 1028 padded cols
RC_ROWS = 32           # output rows per slab per chunk
N_CHUNKS = SLAB_ROWS // RC_ROWS   # 4
XT_ROWS = RC_ROWS + 2 * PAD       # 36 input rows per chunk
ST_ROWS = 4            # output rows per slab per supertile
N_ST = RC_ROWS // ST_ROWS         # 8 supertiles per chunk

_CACHE = {}


def _build(repeat=1, variant="full"):
    key = ("nc", repeat, variant)
    if key in _CACHE:
        return _CACHE[key]
    nc = bacc.Bacc("TRN2", debug=False, num_devices=N_CORES)
    f32 = mybir.dt.float32
    bf16 = mybir.dt.bfloat16

    xs = nc.dram_tensor("xs", [128, SLAB_ROWS + 2 * PAD, WP], bf16,
                        kind="ExternalInput")
    wk = nc.dram_tensor("wk", [128, NT * 32], bf16, kind="ExternalInput")
    bs = nc.dram_tensor("bs", [128, 1], f32, kind="ExternalInput")
    out = nc.dram_tensor("out", [HALF, CH, W], f32, kind="ExternalOutput")

    with tile.TileContext(nc) as tc:
        with (
            tc.tile_pool(name="w", bufs=1) as wpool,
            tc.tile_pool(name="x", bufs=2) as xpool,
            tc.tile_pool(name="o", bufs=6) as opool,
            tc.tile_pool(name="ps", bufs=8, space="PSUM") as pspool,
        ):
            wt = wpool.tile([128, NT * 32], bf16)
            nc.sync.dma_start(wt[:], wk.ap())
            bt = wpool.tile([128, 1], f32)
            nc.sync.dma_start(bt[:], bs.ap())

            from contextlib import nullcontext
            rep_ctx = tc.For_i(0, repeat, 1) if repeat > 1 else nullcontext()
            with rep_ctx:
                _body(nc, tc, xs, out, wt, bt, xpool, opool, pspool,
                      variant=variant)

    _dedup_ldweights(nc)
    _thin_pe_incs(nc)
    nc.compile()
    _CACHE[key] = nc
    return nc


def _thin_pe_incs(nc):
    """Drop the per-matmul sem-inc bookkeeping from the PE stream.

    Tile gives every InstMatmult an on_update sem-inc(PE ctr, +1); each inc
    is a serialized EVT_SEM register write (~26 ns) on the PE sequencer --
    ~666 us across 25600 matmuls, which makes the kernel ISSUE-bound.  Only
    ~500 thresholds of that counter are ever waited on (sem-ge-imm, no
    regs), and matmuls complete in pc order, so it is equivalent to inc
    only at the waited-on thresholds with the accumulated value.
    """
    # find the PE counting semaphore: the one updated by InstMatmult
    pe_sem = None
    for f in nc.m.functions:
        for b in f.blocks:
            for ins in b.instructions:
                if type(ins).__name__ == "InstMatmult" and ins.sync_info:
                    for u in ins.sync_info.on_update:
                        if u.update_mode == "sem-inc":
                            pe_sem = u.id
                            break
                if pe_sem is not None:
                    break
            if pe_sem is not None:
                break
    if pe_sem is None:
        return
    # collect all waited thresholds on that semaphore (module-wide)
    thresholds = set()
    for f in nc.m.functions:
        for b in f.blocks:
            for ins in b.instructions:
                si = ins.sync_info
                if not si:
                    continue
                for w in si.on_wait:
                    if w.id == pe_sem:
                        assert w.wait_mode == "sem-ge-imm" and w.wait_reg is None
                        thresholds.add(w.wait_value)
    # rewrite updates: keep inc only when the running count hits a threshold
    count = 0
    pending = 0
    total = sum(
        u.update_value
        for f in nc.m.functions for b in f.blocks for ins in b.instructions
        if ins.sync_info
        for u in ins.sync_info.on_update
        if u.id == pe_sem and u.update_mode == "sem-inc"
    )
    thresholds.add(total)  # final barrier safety
    for f in nc.m.functions:
        for b in f.blocks:
            for ins in b.instructions:
                si = ins.sync_info
                if not si:
                    continue
                ups = list(si.on_update)
                changed = False
                for ui, u in enumerate(ups):
                    if u.id == pe_sem and u.update_mode == "sem-inc":
                        count += u.update_value
                        pending += u.update_value
                        if count in thresholds:
                            if u.update_value != pending:
                                u.update_value = pending
                                u.update_mode = "sem-add-imm"
                            pending = 0
                        else:
                            ups.pop(ui)
                            changed = True
                        break
                if changed:
                    si.on_update = ups
    assert pending == 0, f"last PE inc dropped ({pending} pending)"


def _dedup_ldweights(nc):
    """Drop InstLdweights that reload the identical weights into the same
    PE array tile as the previous load at that tile_position (our h-pair
    matmuls share weights).  Runs after Tile legalization, before
    bacc.compile() (LDWs carry no sync_info yet; matmul waits are moved to
    the surviving most-recent LDW later by move_matmul_waits_to_ldweights,
    which only ever moves a wait earlier on the same engine - sound)."""
    removed = 0
    for f in nc.m.functions:
        for b in f.blocks:
            il = b.instructions
            last = {}
            out = []
            pending = None
            for ins in il:
                nm = type(ins).__name__
                if nm == "InstLdweights" and ins.sync_info is None \
                        and ins.perf_mode is None:
                    ap = ins.ins[0]
                    sig = (ap.memref, ap.offset, str(ap.ap),
                           str(ins.tile_size), ins.is_transpose)
                    pos = ins.tile_position
                    if last.get(pos) == sig:
                        pending = ins
                        removed += 1
                        continue
                    last[pos] = sig
                elif nm == "InstMatmult" and pending is not None:
                    ins.merge_dependencies_from(pending)
                    pending = None
                out.append(ins)
            assert pending is None
            il[:] = out
    return removed


def _body(nc, tc, xs, out, wt, bt, xpool, opool, pspool, variant="full"):
    """Slab-phase-pipelined emission: slab i runs OFF*i rounds behind slab 0,
    so each slab's PSUM drains + DMA-out hide under the other slabs' matmul
    rounds instead of bursting at a supertile boundary."""
    f32 = mybir.dt.float32
    bf16 = mybir.dt.bfloat16
    PER = N_ST * NT                  # rounds per chunk per slab
    OFF = 4                          # slab phase offset in rounds
    TOT = N_CHUNKS * PER
    G = TOT + OFF * (SLABS - 1)
    xts = {}
    ps_cur = [None] * (2 * SLABS)

    NPIECE = 8

    def load_chunk(c, piece=None):
        if piece is None:
            xt = xpool.tile([128, XT_ROWS * WP], bf16, name="xt", tag="xt")
            src = xs.ap()[:, c * RC_ROWS:c * RC_ROWS + XT_ROWS, :]
            nc.sync.dma_start(xt[:], src.rearrange("p r c -> p (r c)"))
            xts[c] = xt
            return
        if piece == 0:
            xts[c] = xpool.tile([128, XT_ROWS * WP], bf16, name="xt",
                                tag="xt")
        r0 = piece * (XT_ROWS // NPIECE)
        r1 = XT_ROWS if piece == NPIECE - 1 else r0 + XT_ROWS // NPIECE
        src = xs.ap()[:, c * RC_ROWS + r0:c * RC_ROWS + r1, :]
        nc.sync.dma_start(xts[c][:, r0 * WP:r1 * WP],
                          src.rearrange("p r c -> p (r c)"))

    if variant == "mmpure":
        # single SBUF tile, no DMA, no drains: times the bare MM+LDW stream
        xt0 = xpool.tile([128, XT_ROWS * WP], bf16, name="xt0", tag="xt")
        nc.vector.memset(xt0[:], 0.0)
        for c in range(N_CHUNKS):
            xts[c] = xt0

    for g in range(G):
        # prefetch chunk c+1 in NPIECE slices spread over this chunk's
        # rounds: early enough to land before first use, small enough not
        # to head-of-line-block the output DMAs sharing the queues.
        if variant != "mmpure" and g % PER >= 16 and (g % PER - 16) % 16 == 0:
            pc = (g % PER - 16) // 16
            cnext = g // PER + 1
            if cnext < N_CHUNKS and pc < NPIECE and (
                    cnext not in xts or pc > 0):
                load_chunk(cnext, piece=pc)
        active = []
        for i in range(SLABS):
            gi = g - OFF * i
            if gi < 0 or gi >= TOT:
                continue
            c, r = divmod(gi, PER)
            rb, t = divmod(r, NT)
            if c not in xts:
                load_chunk(c)
            if t == 0:
                ps_cur[2 * i] = pspool.tile([128, W // 2], f32,
                                            name=f"ps{i}a", tag="ps")
                ps_cur[2 * i + 1] = pspool.tile([128, W // 2], f32,
                                                name=f"ps{i}b", tag="ps")
            active.append((i, c, rb, t))
        # the slab opening a new PSUM group (t==0) waits on its drain;
        # issue it last so the other slabs' matmuls cover that latency
        # (the PE queue is in-order - a blocked head stalls everything).
        active.sort(key=lambda a: a[3] == 0)
        for h in range(2):
            for i, c, rb, t in active:
                # rotate tap order per supertile so group n's first tap
                # equals group n-1's last tap: the dedup pass then drops the
                # 16 boundary LDWEIGHTS, whose un-overlapped ~1.2us train
                # stalled the PE at every PSUM-group turnaround.
                n_grp = c * N_ST + rb
                ta = (t - n_grp) % NT
                dy, dx = TAPS[ta]
                tw = 0 if variant == "fixw" else ta
                lhsT = wt[32 * i:32 * i + 32, tw * 32:tw * 32 + 32]
                for j in range(ST_ROWS):
                    roff = (rb * ST_ROWS + j + dy) * WP + dx
                    rhs = xts[c][32 * i:32 * i + 32,
                                 roff + h * (W // 2):
                                 roff + h * (W // 2) + W // 2]
                    nc.tensor.matmul(
                        ps_cur[2 * i + h][32 * j:32 * j + 32, :],
                        lhsT, rhs,
                        start=(t == 0), stop=(t == NT - 1),
                        tile_position=(32 * i, 32 * j),
                    )
        if variant in ("nodrain", "mmpure"):
            continue
        for i, c, rb, t in active:
            if t == NT - 1:
                r0 = c * RC_ROWS + rb * ST_ROWS
                ot = opool.tile([128, W], f32, name=f"ot{i}", tag="ot")
                # h0 gates the next supertile's first matmuls; drain it on
                # the faster Activation engine (~570ns vs DVE ~690ns).
                for h in range(2):
                    dl, dr = h * (W // 2), (h + 1) * (W // 2)
                    if h == 1:
                        nc.vector.tensor_scalar_add(
                            ot[:, dl:dr], ps_cur[2 * i + h][:], bt[:])
                    else:
                        nc.scalar.activation(
                            ot[:, dl:dr], ps_cur[2 * i + h][:],
                            mybir.ActivationFunctionType.Identity,
                            bias=bt[:], scale=1.0)
                if variant == "noout":
                    continue
                dst = out.ap()[128 * i + r0:128 * i + r0 + ST_ROWS, :, :]
                nc.sync.dma_start(dst.rearrange("r o c -> (r o) c"), ot[:])


def _prep_inputs(x, weight, bias, psi_loc):
    """Host-side sharding/packing. Returns list of per-core in_maps."""
    kern = np.einsum("kxy,ogk->ogxy", np.asarray(psi_loc, np.float32),
                     np.asarray(weight, np.float32))
    k5 = kern[:, :, 1:6, 1:6]                      # [o, i, 5, 5]
    w32 = k5.transpose(2, 3, 1, 0).reshape(NT, 32, 32)   # [t, i, o]
    warr = np.ascontiguousarray(
        w32.transpose(1, 0, 2).reshape(32, NT * 32))     # [i, t*32+o]
    wk = np.tile(warr, (SLABS, 1)).astype(ml_dtypes.bfloat16)
    bs = np.tile(np.asarray(bias, np.float32), ST_ROWS).reshape(128, 1)

    xb = np.asarray(x, np.float32).astype(ml_dtypes.bfloat16)
    xp = np.pad(xb, ((0, 0), (0, 0), (PAD, PAD), (PAD, PAD)))
    in_maps = []
    for c in range(N_CORES):
        b, hh = divmod(c, 2)
        xcore = xp[b, :, hh * HALF:hh * HALF + HALF + 2 * PAD, :]
        slabs = [xcore[:, SLAB_ROWS * s:SLAB_ROWS * s + SLAB_ROWS + 2 * PAD, :]
                 for s in range(SLABS)]
        xsl = np.ascontiguousarray(
            np.stack(slabs, 0).reshape(128, SLAB_ROWS + 2 * PAD, WP))
        in_maps.append({"xs": xsl, "wk": wk, "bs": bs})
    return in_maps


def run_on_hw(in_maps, **kw):
    nc = _build()
    return run_bass_kernel_spmd(nc, in_maps, core_ids=list(range(N_CORES)), **kw)


def _gather(res):
    full = np.empty((B, CH, H, W), dtype=np.float32)
    for c in range(N_CORES):
        b, hh = divmod(c, 2)
        o = res.results[c]["out"]                  # [512, 32, 1024]
        full[b, :, hh * HALF:(hh + 1) * HALF, :] = o.transpose(1, 0, 2)
    return full


def kernel(x, weight, bias, psi_loc):
    in_maps = _prep_inputs(x, weight, bias, psi_loc)
    res = run_on_hw(in_maps)
    return _gather(res)



# revision 25
# speedup vs baseline: 1.1159x; 1.1159x over previous
"""Equidistant DISCO conv2d (effectively a 5x5 conv, 32->32 ch, 4x1024x1024)
on 8 trn2 NeuronCores.

Sharding: core c handles (batch b = c//2, image half hh = c%2) -> 512 rows.
Inside a core the 512 output rows are split into 4 slabs of 128 rows packed
into SBUF partition groups (partition p = 32*slab + in_ch), so each tap of
the conv runs as 16 concurrent 32x32 matmuls via PE tile_position packing
(4 row-groups = slabs x 4 col-groups = output rows), accumulating 25 taps
into PSUM. bf16 operands, fp32 PSUM accumulation.

The 7x7 DISCO kernel's outer ring is exactly zero (radial hat function hits
zero at r == cutoff), so only the inner 5x5 = 25 taps are computed.
"""

import sys

sys.path.insert(0, "/opt/trn_rl_repo")

import ml_dtypes
import numpy as np

import concourse.bacc as bacc
import concourse.tile as tile
from concourse import mybir
from concourse.bass_utils import run_bass_kernel_spmd

# ---- static geometry ----
B, CH, H, W = 4, 32, 1024, 1024
N_CORES = 8
HALF = H // 2          # rows per core
SLABS = 4
SLAB_ROWS = HALF // SLABS   # 128
PAD = 2                # effective 5x5 kernel
KS = 5
TAPS = [(dy, dx) for dy in range(KS) for dx in range(KS)]
NT = len(TAPS)         # 25
WP = W + 2 * PAD       # 1028 padded cols
RC_ROWS = 32           # output rows per slab per chunk
N_CHUNKS = SLAB_ROWS // RC_ROWS   # 4
XT_ROWS = RC_ROWS + 2 * PAD       # 36 input rows per chunk
ST_ROWS = 4            # output rows per slab per supertile
N_ST = RC_ROWS // ST_ROWS         # 8 supertiles per chunk

_CACHE = {}


def _build(repeat=1, variant="full"):
    key = ("nc", repeat, variant)
    if key in _CACHE:
        return _CACHE[key]
    nc = bacc.Bacc("TRN2", debug=False, num_devices=N_CORES)
    f32 = mybir.dt.float32
    bf16 = mybir.dt.bfloat16

    xs = nc.dram_tensor("xs", [128, SLAB_ROWS + 2 * PAD, WP], bf16,
                        kind="ExternalInput")
    wk = nc.dram_tensor("wk", [128, NT * 32], bf16, kind="ExternalInput")
    bs = nc.dram_tensor("bs", [128, 1], f32, kind="ExternalInput")
    # bf16 output: halves the output DMA traffic (rel-err budget is 2e-2,
    # bf16 rounding adds ~4e-3 worst-case); host converts back to f32.
    out = nc.dram_tensor("out", [HALF, CH, W], bf16, kind="ExternalOutput")

    with tile.TileContext(nc) as tc:
        with (
            tc.tile_pool(name="w", bufs=1) as wpool,
            tc.tile_pool(name="x", bufs=2) as xpool,
            tc.tile_pool(name="o", bufs=6) as opool,
            tc.tile_pool(name="ps", bufs=8, space="PSUM") as pspool,
        ):
            wt = wpool.tile([128, NT * 32], bf16)
            nc.sync.dma_start(wt[:], wk.ap())
            bt = wpool.tile([128, 1], f32)
            nc.sync.dma_start(bt[:], bs.ap())

            from contextlib import nullcontext
            rep_ctx = tc.For_i(0, repeat, 1) if repeat > 1 else nullcontext()
            with rep_ctx:
                _body(nc, tc, xs, out, wt, bt, xpool, opool, pspool,
                      variant=variant)

    _dedup_ldweights(nc)
    _thin_pe_incs(nc)
    nc.compile()
    _CACHE[key] = nc
    return nc


def _thin_pe_incs(nc):
    """Drop the per-matmul sem-inc bookkeeping from the PE stream.

    Tile gives every InstMatmult an on_update sem-inc(PE ctr, +1); each inc
    is a serialized EVT_SEM register write (~26 ns) on the PE sequencer --
    ~666 us across 25600 matmuls, which makes the kernel ISSUE-bound.  Only
    ~500 thresholds of that counter are ever waited on (sem-ge-imm, no
    regs), and matmuls complete in pc order, so it is equivalent to inc
    only at the waited-on thresholds with the accumulated value.
    """
    # find the PE counting semaphore: the one updated by InstMatmult
    pe_sem = None
    for f in nc.m.functions:
        for b in f.blocks:
            for ins in b.instructions:
                if type(ins).__name__ == "InstMatmult" and ins.sync_info:
                    for u in ins.sync_info.on_update:
                        if u.update_mode == "sem-inc":
                            pe_sem = u.id
                            break
                if pe_sem is not None:
                    break
            if pe_sem is not None:
                break
    if pe_sem is None:
        return
    # collect all waited thresholds on that semaphore (module-wide)
    thresholds = set()
    for f in nc.m.functions:
        for b in f.blocks:
            for ins in b.instructions:
                si = ins.sync_info
                if not si:
                    continue
                for w in si.on_wait:
                    if w.id == pe_sem:
                        assert w.wait_mode == "sem-ge-imm" and w.wait_reg is None
                        thresholds.add(w.wait_value)
    # rewrite updates: keep inc only when the running count hits a threshold
    count = 0
    pending = 0
    total = sum(
        u.update_value
        for f in nc.m.functions for b in f.blocks for ins in b.instructions
        if ins.sync_info
        for u in ins.sync_info.on_update
        if u.id == pe_sem and u.update_mode == "sem-inc"
    )
    thresholds.add(total)  # final barrier safety
    for f in nc.m.functions:
        for b in f.blocks:
            for ins in b.instructions:
                si = ins.sync_info
                if not si:
                    continue
                ups = list(si.on_update)
                changed = False
                for ui, u in enumerate(ups):
                    if u.id == pe_sem and u.update_mode == "sem-inc":
                        count += u.update_value
                        pending += u.update_value
                        if count in thresholds:
                            if u.update_value != pending:
                                u.update_value = pending
                                u.update_mode = "sem-add-imm"
                            pending = 0
                        else:
                            ups.pop(ui)
                            changed = True
                        break
                if changed:
                    si.on_update = ups
    assert pending == 0, f"last PE inc dropped ({pending} pending)"


def _dedup_ldweights(nc):
    """Drop InstLdweights that reload the identical weights into the same
    PE array tile as the previous load at that tile_position (our h-pair
    matmuls share weights).  Runs after Tile legalization, before
    bacc.compile() (LDWs carry no sync_info yet; matmul waits are moved to
    the surviving most-recent LDW later by move_matmul_waits_to_ldweights,
    which only ever moves a wait earlier on the same engine - sound)."""
    removed = 0
    for f in nc.m.functions:
        for b in f.blocks:
            il = b.instructions
            last = {}
            out = []
            pending = None
            for ins in il:
                nm = type(ins).__name__
                if nm == "InstLdweights" and ins.sync_info is None \
                        and ins.perf_mode is None:
                    ap = ins.ins[0]
                    sig = (ap.memref, ap.offset, str(ap.ap),
                           str(ins.tile_size), ins.is_transpose)
                    pos = ins.tile_position
                    if last.get(pos) == sig:
                        pending = ins
                        removed += 1
                        continue
                    last[pos] = sig
                elif nm == "InstMatmult" and pending is not None:
                    ins.merge_dependencies_from(pending)
                    pending = None
                out.append(ins)
            assert pending is None
            il[:] = out
    return removed


def _body(nc, tc, xs, out, wt, bt, xpool, opool, pspool, variant="full"):
    """Slab-phase-pipelined emission: slab i runs OFF*i rounds behind slab 0,
    so each slab's PSUM drains + DMA-out hide under the other slabs' matmul
    rounds instead of bursting at a supertile boundary."""
    f32 = mybir.dt.float32
    bf16 = mybir.dt.bfloat16
    PER = N_ST * NT                  # rounds per chunk per slab
    OFF = 4                          # slab phase offset in rounds
    TOT = N_CHUNKS * PER
    G = TOT + OFF * (SLABS - 1)
    xts = {}
    ps_cur = [None] * (2 * SLABS)

    NPIECE = 8

    def load_chunk(c, piece=None):
        if piece is None:
            xt = xpool.tile([128, XT_ROWS * WP], bf16, name="xt", tag="xt")
            src = xs.ap()[:, c * RC_ROWS:c * RC_ROWS + XT_ROWS, :]
            nc.sync.dma_start(xt[:], src.rearrange("p r c -> p (r c)"))
            xts[c] = xt
            return
        if piece == 0:
            xts[c] = xpool.tile([128, XT_ROWS * WP], bf16, name="xt",
                                tag="xt")
        r0 = piece * (XT_ROWS // NPIECE)
        r1 = XT_ROWS if piece == NPIECE - 1 else r0 + XT_ROWS // NPIECE
        src = xs.ap()[:, c * RC_ROWS + r0:c * RC_ROWS + r1, :]
        nc.sync.dma_start(xts[c][:, r0 * WP:r1 * WP],
                          src.rearrange("p r c -> p (r c)"))

    if variant == "mmpure":
        # single SBUF tile, no DMA, no drains: times the bare MM+LDW stream
        xt0 = xpool.tile([128, XT_ROWS * WP], bf16, name="xt0", tag="xt")
        nc.vector.memset(xt0[:], 0.0)
        for c in range(N_CHUNKS):
            xts[c] = xt0

    for g in range(G):
        # prefetch chunk c+1 in NPIECE slices spread over this chunk's
        # rounds: early enough to land before first use, small enough not
        # to head-of-line-block the output DMAs sharing the queues.
        if variant != "mmpure" and g % PER >= 16 and (g % PER - 16) % 16 == 0:
            pc = (g % PER - 16) // 16
            cnext = g // PER + 1
            if cnext < N_CHUNKS and pc < NPIECE and (
                    cnext not in xts or pc > 0):
                load_chunk(cnext, piece=pc)
        active = []
        for i in range(SLABS):
            gi = g - OFF * i
            if gi < 0 or gi >= TOT:
                continue
            c, r = divmod(gi, PER)
            rb, t = divmod(r, NT)
            if c not in xts:
                if c == 0:
                    # piecewise: xt has subtile deps, so the first
                    # supertiles' matmuls start after ~3 pieces (~9us)
                    # instead of waiting out the whole 9.5MB load (~26us)
                    for p in range(NPIECE):
                        load_chunk(0, piece=p)
                else:
                    load_chunk(c)
            if t == 0:
                ps_cur[2 * i] = pspool.tile([128, W // 2], f32,
                                            name=f"ps{i}a", tag="ps")
                ps_cur[2 * i + 1] = pspool.tile([128, W // 2], f32,
                                                name=f"ps{i}b", tag="ps")
            active.append((i, c, rb, t))
        # the slab opening a new PSUM group (t==0) waits on its drain;
        # issue it last so the other slabs' matmuls cover that latency
        # (the PE queue is in-order - a blocked head stalls everything).
        active.sort(key=lambda a: a[3] == 0)
        for h in range(2):
            for i, c, rb, t in active:
                # NOTE: rotating tap order per supertile to dedup the
                # boundary LDW was tried and is HARMFUL: the legalizer moves
                # the start-MM's psum-drain wait onto the surviving LDW in
                # the PREVIOUS round, stalling the previous group's tail.
                dy, dx = TAPS[t]
                tw = 0 if variant == "fixw" else t
                lhsT = wt[32 * i:32 * i + 32, tw * 32:tw * 32 + 32]
                for j in range(ST_ROWS):
                    roff = (rb * ST_ROWS + j + dy) * WP + dx
                    rhs = xts[c][32 * i:32 * i + 32,
                                 roff + h * (W // 2):
                                 roff + h * (W // 2) + W // 2]
                    nc.tensor.matmul(
                        ps_cur[2 * i + h][32 * j:32 * j + 32, :],
                        lhsT, rhs,
                        start=(t == 0), stop=(t == NT - 1),
                        tile_position=(32 * i, 32 * j),
                    )
        if variant in ("nodrain", "mmpure"):
            continue
        for i, c, rb, t in active:
            if t == NT - 1:
                r0 = c * RC_ROWS + rb * ST_ROWS
                ot = opool.tile([128, W], bf16, name=f"ot{i}", tag="ot")
                for h in range(2):
                    dl, dr = h * (W // 2), (h + 1) * (W // 2)
                    if h == 0:
                        nc.vector.tensor_scalar_add(
                            ot[:, dl:dr], ps_cur[2 * i + h][:], bt[:])
                    else:
                        nc.scalar.activation(
                            ot[:, dl:dr], ps_cur[2 * i + h][:],
                            mybir.ActivationFunctionType.Identity,
                            bias=bt[:], scale=1.0)
                if variant == "noout":
                    continue
                dst = out.ap()[128 * i + r0:128 * i + r0 + ST_ROWS, :, :]
                nc.sync.dma_start(dst.rearrange("r o c -> (r o) c"), ot[:])


def _prep_inputs(x, weight, bias, psi_loc):
    """Host-side sharding/packing. Returns list of per-core in_maps."""
    kern = np.einsum("kxy,ogk->ogxy", np.asarray(psi_loc, np.float32),
                     np.asarray(weight, np.float32))
    k5 = kern[:, :, 1:6, 1:6]                      # [o, i, 5, 5]
    w32 = k5.transpose(2, 3, 1, 0).reshape(NT, 32, 32)   # [t, i, o]
    warr = np.ascontiguousarray(
        w32.transpose(1, 0, 2).reshape(32, NT * 32))     # [i, t*32+o]
    wk = np.tile(warr, (SLABS, 1)).astype(ml_dtypes.bfloat16)
    bs = np.tile(np.asarray(bias, np.float32), ST_ROWS).reshape(128, 1)

    xb = np.asarray(x, np.float32).astype(ml_dtypes.bfloat16)
    xp = np.pad(xb, ((0, 0), (0, 0), (PAD, PAD), (PAD, PAD)))
    in_maps = []
    for c in range(N_CORES):
        b, hh = divmod(c, 2)
        xcore = xp[b, :, hh * HALF:hh * HALF + HALF + 2 * PAD, :]
        slabs = [xcore[:, SLAB_ROWS * s:SLAB_ROWS * s + SLAB_ROWS + 2 * PAD, :]
                 for s in range(SLABS)]
        xsl = np.ascontiguousarray(
            np.stack(slabs, 0).reshape(128, SLAB_ROWS + 2 * PAD, WP))
        in_maps.append({"xs": xsl, "wk": wk, "bs": bs})
    return in_maps


def run_on_hw(in_maps, **kw):
    nc = _build()
    return run_bass_kernel_spmd(nc, in_maps, core_ids=list(range(N_CORES)), **kw)


def _gather(res):
    full = np.empty((B, CH, H, W), dtype=np.float32)
    for c in range(N_CORES):
        b, hh = divmod(c, 2)
        o = res.results[c]["out"]                  # [512, 32, 1024] bf16
        full[b, :, hh * HALF:(hh + 1) * HALF, :] = \
            o.transpose(1, 0, 2).astype(np.float32)
    return full


def kernel(x, weight, bias, psi_loc):
    in_maps = _prep_inputs(x, weight, bias, psi_loc)
    res = run_on_hw(in_maps)
    return _gather(res)

